# revision 1
# baseline (speedup 1.0000x reference)
"""Trainium2 Bass kernel for nn_NodePreTrans (e3nn tensor product + linear).

v3 design "T": all-bf16 datapath (x, weights, p-tiles, stores), f32 PSUM.

Engine budget per 512-col z-block (measured rates):
  PE  : 39 MMs (36 + 3 extra: p5 sub folded into o1e via negated L1e)
  DVE : 3 mixed muls (sbuf x psum, 1x) + A/B/p2g4/p4-adds at bf16 2x
  ACT : Ewc copy, g4 copy, 2 wide evacs, store trigger (all 2048-wide)
  GpS : p3 only (bf16, slow but fits the shared-port budget with DVE-2x)
Port rules: DVE-2x (two SBUF reads) and GpSimd exclude each other; DVE
mixed/psum ops don't touch the shared pair.
PSUM: one pool of [128, 2048] grabs (4 banks) x bufs=2 = 8 banks.
outT row order: [o0e(2) | o1e(3) | o1o(3)] so both stage-2 evacs are
contiguous 2048-wide copies.
"""

import sys

sys.path.insert(0, "/opt/trn_rl_repo")

import numpy as np

import concourse.bacc as bacc
import concourse.bass as bass
import concourse.mybir as mybir
import concourse.tile as tile
from concourse.bass_utils import run_bass_kernel_spmd

N_NODES = 50000
N_CORES = 8
NS = N_NODES // N_CORES
NSH = 6272                       # 12*512 + 128
TW = 512

C_000 = 1.0 / np.sqrt(256.0)
C_011 = 1.0 / np.sqrt(128.0)
C_101 = 1.0 / np.sqrt(256.0)
C_110 = 1.0 / np.sqrt(384.0)
C_111 = 1.0 / 16.0

F32 = mybir.dt.float32
BF16 = mybir.dt.bfloat16

_CACHE = {}


def _ap3(ap2, n, w):
    """View a 2D contiguous AP [P, n*w] as [P, n, w]."""
    (ps, pn), (s, c) = ap2.ap[0], ap2.ap[1]
    assert s == 1 and c == n * w, (ap2.ap, n, w)
    return bass.AP(ap2.tensor, ap2.offset, [(ps, pn), (w, n), (1, w)])


def _bcast(ap2, n):
    """Broadcast a 2D AP [P, w] to [P, n, w] with a stride-0 dim."""
    (ps, pn), (s, c) = ap2.ap[0], ap2.ap[1]
    return bass.AP(ap2.tensor, ap2.offset, [(ps, pn), (0, n), (s, c)])


def _dram3(dram_ap, row0, n, z0, Z, nsh=NSH):
    """[128, n, Z] view over dram [rows, NSH]; row = row0 + 128*chunk + p."""
    base = dram_ap[row0:row0 + 128, z0:z0 + Z]
    return bass.AP(base.tensor, base.offset,
                   [(nsh, 128), (128 * nsh, n), (1, Z)])


def _build_program():
    nc = bacc.Bacc("TRN2", target_bir_lowering=False, debug=False,
                   num_devices=N_CORES)

    xT_d = nc.dram_tensor("xT", [640, NSH], BF16, kind="ExternalInput").ap()
    wt000_d = nc.dram_tensor("wt000", [256, 256], BF16, kind="ExternalInput").ap()
    wt011_d = nc.dram_tensor("wt011", [128, 256], BF16, kind="ExternalInput").ap()
    wt101_d = nc.dram_tensor("wt101", [256, 128], BF16, kind="ExternalInput").ap()
    wt110_d = nc.dram_tensor("wt110", [128, 128], BF16, kind="ExternalInput").ap()
    wt111_d = nc.dram_tensor("wt111", [128, 128], BF16, kind="ExternalInput").ap()
    l0e_d = nc.dram_tensor("l0e", [384, 256], BF16, kind="ExternalInput").ap()
    l1o_d = nc.dram_tensor("l1o", [384, 128], BF16, kind="ExternalInput").ap()
    l1e_d = nc.dram_tensor("l1e", [128, 256], BF16, kind="ExternalInput").ap()
    outT_d = nc.dram_tensor("outT", [1024, NSH], BF16, kind="ExternalOutput").ap()

    with tile.TileContext(nc) as tc:
        _emit(tc, nc, xT_d, wt000_d, wt011_d, wt101_d, wt110_d, wt111_d,
              l0e_d, l1o_d, l1e_d, outT_d)

    nc.compile()
    return nc


def _emit(tc, nc, xT_d, wt000_d, wt011_d, wt101_d, wt110_d, wt111_d,
          l0e_d, l1o_d, l1e_d, outT_d):
    zblocks = [(0, 256), (256, 256)] + \
        [(512 + i * TW, TW) for i in range(11)] + [(6144, 128)]
    with (
        tc.tile_pool(name="wpool", bufs=1) as wpool,
        tc.tile_pool(name="xin", bufs=4) as xin,
        tc.tile_pool(name="mid", bufs=3) as mid,
        tc.tile_pool(name="pt", bufs=3) as pt,
        tc.tile_pool(name="ost", bufs=3) as ost,
        tc.tile_pool(name="ps1", bufs=1, space="PSUM") as ps1,
    ):
        # warmup MMs keep PE busy (HAM un-throttle) while first loads land
        junk = wpool.tile([128, 256], BF16, name="junk")
        nc.gpsimd.memset(junk[:, :], 0.125)
        wps = ps1.tile([128, 2 * TW], F32, name="s1", bufs=4)
        for _ in range(38):
            nc.tensor.matmul(wps[:, :128], junk[:, 0:128], junk[:, 128:256],
                             start=True, stop=True)

        def wtile(name, dram_ap, rows, cols):
            t = wpool.tile([128, cols], BF16, name=name)
            nc.scalar.dma_start(t[:, :], dram_ap[rows:rows + 128, :])
            return t

        w111 = wtile("w111", wt111_d, 0, 128)
        w101 = [wtile(f"w101_{k}", wt101_d, 128 * k, 128) for k in range(2)]
        w000 = [wtile(f"w000_{k}", wt000_d, 128 * k, 256) for k in range(2)]
        w110 = wtile("w110", wt110_d, 0, 128)
        w011 = wtile("w011", wt011_d, 0, 256)
        L0e = [wtile(f"l0e_{k}", l0e_d, 128 * k, 256) for k in range(3)]
        # l1e dram holds [L1e | -L1e] as 256 cols
        L1e2 = wtile("l1e2", l1e_d, 0, 256)
        L1o = [wtile(f"l1o_{k}", l1o_d, 128 * k, 128) for k in range(3)]

        def mm(out, lhsT, rhs, start=True, stop=True):
            nc.tensor.matmul(out, lhsT, rhs, start=start, stop=stop)

        def load(bi):
            z0, Z = zblocks[bi]
            x_w = xin.tile([128, 5 * TW], BF16, name="x_w")
            nc.sync.dma_start(_ap3(x_w[:, :5 * Z], 5, Z),
                              _dram3(xT_d, 0, 5, z0, Z))
            return x_w

        def stage1(bi, x_w):
            z0, Z = zblocks[bi]

            def sx(i, n=1):
                return x_w[:, i * Z:(i + n) * Z]

            def vx(i, n=1):
                return x_w[:, (2 + i) * Z:(2 + i + n) * Z]

            def grab():
                return ps1.tile([128, 2 * TW], F32, name="s1", bufs=4)

            # ---- h3 = [a0|a1] -> p1 (first: DVE head start) ---------------
            h3 = grab()
            for m in range(2):
                mm(h3[:, m * Z:(m + 1) * Z],
                   w000[0][:, 128 * m:128 * (m + 1)], sx(0),
                   start=True, stop=False)
                mm(h3[:, m * Z:(m + 1) * Z],
                   w000[1][:, 128 * m:128 * (m + 1)], sx(1),
                   start=False, stop=True)
            p1 = pt.tile([128, 2 * TW], BF16, name="p1")
            nc.vector.tensor_mul(p1[:, :2 * Z], x_w[:, :2 * Z], h3[:, :2 * Z])

            # ---- h1 = [E1|E2], h2 = [E0|c]  (E rotated) -------------------
            h1 = grab()
            mm(h1[:, 0:Z], w111[:, :], vx(1))
            mm(h1[:, Z:2 * Z], w111[:, :], vx(2))
            h2 = grab()
            mm(h2[:, 0:Z], w111[:, :], vx(0))
            mm(h2[:, Z:2 * Z], w101[0][:, :], sx(0), start=True, stop=False)
            mm(h2[:, Z:2 * Z], w101[1][:, :], sx(1), start=False, stop=True)
            Ewc = mid.tile([128, 4 * TW], BF16, name="Ewc")
            nc.scalar.copy(Ewc[:, 0:2 * Z], h1[:, :2 * Z])
            nc.scalar.copy(Ewc[:, 2 * Z:4 * Z], h2[:, :2 * Z])

            def ew(i, n=1):
                return Ewc[:, i * Z:(i + n) * Z]

            cw = Ewc[:, 3 * Z:4 * Z]

            # ---- h4 = [d0|d1], h5 = [d2|-] -> p4 (early for k1) -----------
            h4 = grab()
            mm(h4[:, 0:Z], w110[:, :], vx(0))
            mm(h4[:, Z:2 * Z], w110[:, :], vx(1))
            p4t = pt.tile([128, 3 * TW], BF16, name="p4t")
            nc.vector.tensor_mul(p4t[:, :2 * Z], vx(0, 2), h4[:, :2 * Z])
            h5 = grab()
            mm(h5[:, 0:Z], w110[:, :], vx(2))
            p4t2 = pt.tile([128, TW], BF16, name="p4t2")
            nc.vector.tensor_mul(p4t2[:, :Z], vx(2), h5[:, 0:Z])
            p4 = pt.tile([128, TW], BF16, name="p4")
            nc.gpsimd.tensor_add(p4[:, :Z], p4t[:, :Z], p4t[:, Z:2 * Z])
            nc.gpsimd.tensor_add(p4[:, :Z], p4[:, :Z], p4t2[:, :Z])

            # ---- GpS: p3 = v * bcast(c) -----------------------------------
            p3 = pt.tile([128, 3 * TW], BF16, name="p3")
            nc.gpsimd.tensor_mul(_ap3(p3[:, :3 * Z], 3, Z),
                                 _ap3(x_w[:, 2 * Z:5 * Z], 3, Z),
                                 _bcast(cw[:, :Z], 3))

            # ---- h6..h8 = [b_j0|b_j1] -> p2_j -----------------------------
            p2 = pt.tile([128, 6 * TW], BF16, name="p2")
            for j in range(3):
                h = grab()
                mm(h[:, 0:Z], w011[:, 0:128], vx(j))
                mm(h[:, Z:2 * Z], w011[:, 128:256], vx(j))
                nc.vector.tensor_mul(p2[:, j * 2 * Z:(j + 1) * 2 * Z],
                                     x_w[:, :2 * Z], h[:, :2 * Z])

            # ---- DVE 2x: A/B products (late; consumed next step) ----------
            A = mid.tile([128, 3 * TW], BF16, name="A")
            nc.vector.tensor_mul(A[:, 0:2 * Z], vx(1, 2), ew(1, 2))
            nc.vector.tensor_mul(A[:, 2 * Z:3 * Z], vx(0), ew(0))
            B = mid.tile([128, 3 * TW], BF16, name="B")
            nc.vector.tensor_mul(B[:, 0:Z], vx(2), ew(0))
            nc.vector.tensor_mul(B[:, Z:3 * Z], vx(0, 2), ew(1, 2))
            return dict(z0=z0, Z=Z, p1=p1, p4=p4, p2=p2, p3=p3, A=A, B=B)

        def stage2(st, split_store=False):
            z0, Z = st["z0"], st["Z"]
            p1, p4, p2, p3, A, B = (st["p1"], st["p4"], st["p2"], st["p3"],
                                    st["A"], st["B"])
            stor = ost.tile([128, 8 * TW], BF16, name="stor")

            def grab():
                return ps1.tile([128, 2 * TW], F32, name="s1", bufs=4)

            k1 = grab()
            for m in range(2):
                o = k1[:, m * Z:(m + 1) * Z]
                mm(o, L0e[0][:, 128 * m:128 * (m + 1)], p1[:, :Z],
                   start=True, stop=False)
                mm(o, L0e[1][:, 128 * m:128 * (m + 1)], p1[:, Z:2 * Z],
                   start=False, stop=False)
                mm(o, L0e[2][:, 128 * m:128 * (m + 1)], p4[:, :Z],
                   start=False, stop=True)
            nc.scalar.copy(stor[:, 0:2 * Z], k1[:, :2 * Z])
            if split_store:
                nc.sync.dma_start(_dram3(outT_d, 0, 2, z0, Z),
                                  _ap3(stor[:, 0:2 * Z], 2, Z))
            k2 = grab()
            for j in range(2):
                o = k2[:, j * Z:(j + 1) * Z]
                mm(o, L1e2[:, 0:128], A[:, j * Z:(j + 1) * Z],
                   start=True, stop=False)
                mm(o, L1e2[:, 128:256], B[:, j * Z:(j + 1) * Z],
                   start=False, stop=True)
            nc.scalar.copy(stor[:, 2 * Z:4 * Z], k2[:, :2 * Z])
            if split_store:
                nc.sync.dma_start(_dram3(outT_d, 256, 2, z0, Z),
                                  _ap3(stor[:, 2 * Z:4 * Z], 2, Z))

            def o1o(o, j):
                mm(o, L1o[0][:, :], p2[:, j * 2 * Z:j * 2 * Z + Z],
                   start=True, stop=False)
                mm(o, L1o[1][:, :], p2[:, j * 2 * Z + Z:(j + 1) * 2 * Z],
                   start=False, stop=False)
                mm(o, L1o[2][:, :], p3[:, j * Z:(j + 1) * Z],
                   start=False, stop=True)

            k3 = grab()
            mm(k3[:, 0:Z], L1e2[:, 0:128], A[:, 2 * Z:3 * Z],
               start=True, stop=False)
            mm(k3[:, 0:Z], L1e2[:, 128:256], B[:, 2 * Z:3 * Z],
               start=False, stop=True)
            o1o(k3[:, Z:2 * Z], 0)
            nc.scalar.copy(stor[:, 4 * Z:6 * Z], k3[:, :2 * Z])
            if split_store:
                nc.sync.dma_start(_dram3(outT_d, 512, 2, z0, Z),
                                  _ap3(stor[:, 4 * Z:6 * Z], 2, Z))
            k4 = grab()
            o1o(k4[:, 0:Z], 1)
            o1o(k4[:, Z:2 * Z], 2)
            nc.scalar.copy(stor[:, 6 * Z:8 * Z], k4[:, :2 * Z])
            if split_store:
                nc.sync.dma_start(_dram3(outT_d, 768, 2, z0, Z),
                                  _ap3(stor[:, 6 * Z:8 * Z], 2, Z))
                return None
            return dict(z0=z0, Z=Z, stor=stor)

        def store(st):
            z0, Z, stor = st["z0"], st["Z"], st["stor"]
            nc.sync.dma_start(_dram3(outT_d, 0, 8, z0, Z),
                              _ap3(stor[:, :8 * Z], 8, Z))

        # software pipeline with 2-deep load prefetch
        nblk = len(zblocks)
        xq = [load(0), load(1)]
        pend1 = None
        pend2 = None
        for bi in range(nblk):
            st = stage1(bi, xq.pop(0))
            if pend2 is not None:
                store(pend2)
            if pend1 is not None:
                pend2 = stage2(pend1)
            pend1 = st
            if bi + 2 < nblk:
                xq.append(load(bi + 2))
        store(pend2)
        stage2(pend1, split_store=True)


def _prep_inputs(node_feat, w_00_0, w_01_1, w_10_1, w_11_0, w_11_1,
                 W_0e, W_1o, W_1e):
    import ml_dtypes
    ndt = ml_dtypes.bfloat16
    l1e = W_1e / np.sqrt(128.0)
    l1e2 = np.concatenate([l1e, -l1e], axis=1)        # [128, 256]
    weights = {
        "wt000": np.ascontiguousarray((C_000 * w_00_0).T).astype(ndt),
        "wt011": np.ascontiguousarray((C_011 * w_01_1).T).astype(ndt),
        "wt101": np.ascontiguousarray((C_101 * w_10_1).T).astype(ndt),
        "wt110": np.ascontiguousarray((C_110 * w_11_0).T).astype(ndt),
        "wt111": np.ascontiguousarray((C_111 * w_11_1).T).astype(ndt),
        "l0e": np.ascontiguousarray(W_0e / np.sqrt(384.0)).astype(ndt),
        "l1o": np.ascontiguousarray(W_1o / np.sqrt(384.0)).astype(ndt),
        "l1e": np.ascontiguousarray(l1e2).astype(ndt),
    }
    feat = np.asarray(node_feat, dtype=np.float32).reshape(N_CORES, NS, 640)
    in_maps = []
    for i in range(N_CORES):
        blk = feat[i]
        xT = np.zeros((640, NSH), ndt)
        xT[:256, :NS] = blk[:, :256].T.astype(ndt)
        vv = blk[:, 256:].reshape(NS, 128, 3)
        xT[256:, :NS] = vv.transpose(2, 1, 0).reshape(384, NS).astype(ndt)
        in_maps.append({"xT": xT, **weights})
    return in_maps


def _gather(results):
    # outT rows: [o0e(0:256) | o1e(256:640) | o1o(640:1024)]
    out = np.empty((N_NODES, 1024), np.float32)
    for i in range(N_CORES):
        oT = np.asarray(results[i]["outT"]).astype(np.float32,
                                                   copy=False)[:, :NS]
        blk = out[i * NS:(i + 1) * NS]
        blk[:, :256] = oT[:256].T
        blk[:, 640:] = oT[256:640].reshape(3, 128, NS).transpose(2, 1, 0) \
            .reshape(NS, 384)
        blk[:, 256:640] = oT[640:].reshape(3, 128, NS).transpose(2, 1, 0) \
            .reshape(NS, 384)
    return out


def kernel(node_feat, w_00_0, w_01_1, w_10_1, w_11_0, w_11_1,
           W_0e, W_1o, W_1e, _trace=False):
    if "v3" not in _CACHE:
        _CACHE["v3"] = _build_program()
    nc = _CACHE["v3"]
    in_maps = _prep_inputs(node_feat, w_00_0, w_01_1, w_10_1, w_11_0,
                           w_11_1, W_0e, W_1o, W_1e)
    res = run_bass_kernel_spmd(nc, in_maps, core_ids=list(range(N_CORES)),
                               trace=_trace)
    out = _gather(res.results)
    if _trace:
        return out, res
    return out



# revision 2
# speedup vs baseline: 1.0036x; 1.0036x over previous
"""Trainium2 Bass kernel for nn_NodePreTrans (e3nn tensor product + linear).

v6: v3 "T" design plus
  - weights packed into ONE dram tensor [128, 2688], DMA'd as two chunks
    (stage1 cols first) on the scalar queue at t=0 (v3 issued 10 serial
    triggers; first real MM ~15.8us)
  - x loads split into s-part [2 chunks] and v-part [3 chunks] so the
    h3 matmuls (s-only) can start before the v rows land; sync queue
    carries loads only
  - stores: triggered inline from stage2 via the Sync engine (idle, so
    it absorbs DMA-ring backpressure waits; scalar triggers block ACT's
    FIFO ~17us/block -- v6 regression); the last two blocks split their
    store across both queues (qAct is empty by then) for parallel drain
  - warmup: 30 MMs (~3.2us continuous) so HAM un-throttles before the
    first real MM and there is no >3.4us PE idle at the handoff
  - DVE TT ops kept <=1024 cols (1536-col ops measured 1.5 cyc/elem,
    the 2x uop path stops at 1024)
  - p1 stays DVE-mixed (v4 tried ACT-evac+DVE-2x: ACT strict-FIFO
    head-of-line blocking stalled DVE ~0.4us/block -- reverted)
"""

import sys

sys.path.insert(0, "/opt/trn_rl_repo")

import numpy as np

import concourse.bacc as bacc
import concourse.bass as bass
import concourse.mybir as mybir
import concourse.tile as tile
from concourse.bass_utils import run_bass_kernel_spmd

N_NODES = 50000
N_CORES = 8
NS = N_NODES // N_CORES
NSH = 6272                       # 12*512 + 128
TW = 512

C_000 = 1.0 / np.sqrt(256.0)
C_011 = 1.0 / np.sqrt(128.0)
C_101 = 1.0 / np.sqrt(256.0)
C_110 = 1.0 / np.sqrt(384.0)
C_111 = 1.0 / 16.0

F32 = mybir.dt.float32
BF16 = mybir.dt.bfloat16

_CACHE = {}

# column layout of the packed weights tensor [128, 2688]
WCOL = {
    "w111": (0, 128),
    "w101_0": (128, 128), "w101_1": (256, 128),
    "w000_0": (384, 256), "w000_1": (640, 256),
    "w110": (896, 128),
    "w011": (1024, 256),
    "l0e_0": (1280, 256), "l0e_1": (1536, 256), "l0e_2": (1792, 256),
    "l1e2": (2048, 256),
    "l1o_0": (2304, 128), "l1o_1": (2432, 128), "l1o_2": (2560, 128),
}
WALL_COLS = 2688


def _ap3(ap2, n, w):
    """View a 2D contiguous AP [P, n*w] as [P, n, w]."""
    (ps, pn), (s, c) = ap2.ap[0], ap2.ap[1]
    assert s == 1 and c == n * w, (ap2.ap, n, w)
    return bass.AP(ap2.tensor, ap2.offset, [(ps, pn), (w, n), (1, w)])


def _bcast(ap2, n):
    """Broadcast a 2D AP [P, w] to [P, n, w] with a stride-0 dim."""
    (ps, pn), (s, c) = ap2.ap[0], ap2.ap[1]
    return bass.AP(ap2.tensor, ap2.offset, [(ps, pn), (0, n), (s, c)])


def _dram3(dram_ap, row0, n, z0, Z, nsh=NSH):
    """[128, n, Z] view over dram [rows, NSH]; row = row0 + 128*chunk + p."""
    base = dram_ap[row0:row0 + 128, z0:z0 + Z]
    return bass.AP(base.tensor, base.offset,
                   [(nsh, 128), (128 * nsh, n), (1, Z)])


def _build_program():
    nc = bacc.Bacc("TRN2", target_bir_lowering=False, debug=False,
                   num_devices=N_CORES)

    xT_d = nc.dram_tensor("xT", [640, NSH], BF16, kind="ExternalInput").ap()
    wall_d = nc.dram_tensor("wall", [128, WALL_COLS], BF16,
                            kind="ExternalInput").ap()
    outT_d = nc.dram_tensor("outT", [1024, NSH], BF16, kind="ExternalOutput").ap()

    with tile.TileContext(nc) as tc:
        _emit(tc, nc, xT_d, wall_d, outT_d)

    nc.compile()
    return nc


def _emit(tc, nc, xT_d, wall_d, outT_d):
    zblocks = [(0, 256), (256, 256)] + \
        [(512 + i * TW, TW) for i in range(11)] + [(6144, 128)]
    with (
        tc.tile_pool(name="wpool", bufs=1) as wpool,
        tc.tile_pool(name="xin", bufs=4) as xin,
        tc.tile_pool(name="mid", bufs=4) as mid,
        tc.tile_pool(name="pt", bufs=3) as pt,
        tc.tile_pool(name="ost", bufs=3) as ost,
        tc.tile_pool(name="ps1", bufs=1, space="PSUM") as ps1,
    ):
        # packed weights: stage1 cols (0:1280) first, stage2 cols behind
        wall = wpool.tile([128, WALL_COLS], BF16, name="wall")
        nc.scalar.dma_start(wall[:, 0:1280], wall_d[:, 0:1280])
        nc.scalar.dma_start(wall[:, 1280:WALL_COLS], wall_d[:, 1280:WALL_COLS])

        def wv(key):
            c0, w = WCOL[key]
            return wall[:, c0:c0 + w]

        w111 = wv("w111")
        w101 = [wv("w101_0"), wv("w101_1")]
        w000 = [wv("w000_0"), wv("w000_1")]
        w110 = wv("w110")
        w011 = wv("w011")
        L0e = [wv("l0e_0"), wv("l0e_1"), wv("l0e_2")]
        L1e2 = wv("l1e2")          # [L1e | -L1e] as 256 cols
        L1o = [wv("l1o_0"), wv("l1o_1"), wv("l1o_2")]

        def load(bi):
            z0, Z = zblocks[bi]
            x_s = xin.tile([128, 2 * TW], BF16, name="x_s", tag="xs")
            nc.sync.dma_start(_ap3(x_s[:, :2 * Z], 2, Z),
                              _dram3(xT_d, 0, 2, z0, Z))
            x_v = xin.tile([128, 3 * TW], BF16, name="x_v", tag="xv")
            nc.sync.dma_start(_ap3(x_v[:, :3 * Z], 3, Z),
                              _dram3(xT_d, 256, 3, z0, Z))
            return (x_s, x_v)

        xq = [load(0), load(1)]

        # warmup MMs keep PE busy (HAM un-throttle) while loads land
        junk = wpool.tile([128, 256], BF16, name="junk")
        nc.gpsimd.memset(junk[:, :], 0.125)
        wps = ps1.tile([128, 2 * TW], F32, name="s1", bufs=4)
        for _ in range(30):
            nc.tensor.matmul(wps[:, :128], junk[:, 0:128], junk[:, 128:256],
                             start=True, stop=True)

        def mm(out, lhsT, rhs, start=True, stop=True):
            nc.tensor.matmul(out, lhsT, rhs, start=start, stop=stop)

        def stage1(bi, xsv):
            z0, Z = zblocks[bi]
            x_s, x_v = xsv

            def sx(i, n=1):
                return x_s[:, i * Z:(i + n) * Z]

            def vx(i, n=1):
                return x_v[:, i * Z:(i + n) * Z]

            def grab():
                return ps1.tile([128, 2 * TW], F32, name="s1", bufs=4)

            # ---- h3 = [a0|a1] -> p1 (first: DVE head start) ---------------
            h3 = grab()
            for m in range(2):
                mm(h3[:, m * Z:(m + 1) * Z],
                   w000[0][:, 128 * m:128 * (m + 1)], sx(0),
                   start=True, stop=False)
                mm(h3[:, m * Z:(m + 1) * Z],
                   w000[1][:, 128 * m:128 * (m + 1)], sx(1),
                   start=False, stop=True)
            p1 = pt.tile([128, 2 * TW], BF16, name="p1")
            nc.vector.tensor_mul(p1[:, :2 * Z], x_s[:, :2 * Z], h3[:, :2 * Z])

            # ---- h1 = [E1|E2], h2 = [E0|c]  (E rotated) -------------------
            h1 = grab()
            mm(h1[:, 0:Z], w111[:, :], vx(1))
            mm(h1[:, Z:2 * Z], w111[:, :], vx(2))
            h2 = grab()
            mm(h2[:, 0:Z], w111[:, :], vx(0))
            mm(h2[:, Z:2 * Z], w101[0][:, :], sx(0), start=True, stop=False)
            mm(h2[:, Z:2 * Z], w101[1][:, :], sx(1), start=False, stop=True)
            Ewc = mid.tile([128, 4 * TW], BF16, name="Ewc")
            nc.scalar.copy(Ewc[:, 0:2 * Z], h1[:, :2 * Z])
            nc.scalar.copy(Ewc[:, 2 * Z:4 * Z], h2[:, :2 * Z])

            def ew(i, n=1):
                return Ewc[:, i * Z:(i + n) * Z]

            cw = Ewc[:, 3 * Z:4 * Z]

            # ---- h4 = [d0|d1], h5 = [d2|-] -> p4 (early for k1) -----------
            h4 = grab()
            mm(h4[:, 0:Z], w110[:, :], vx(0))
            mm(h4[:, Z:2 * Z], w110[:, :], vx(1))
            p4t = pt.tile([128, 3 * TW], BF16, name="p4t")
            nc.vector.tensor_mul(p4t[:, :2 * Z], vx(0, 2), h4[:, :2 * Z])
            h5 = grab()
            mm(h5[:, 0:Z], w110[:, :], vx(2))
            p4t2 = pt.tile([128, TW], BF16, name="p4t2")
            nc.vector.tensor_mul(p4t2[:, :Z], vx(2), h5[:, 0:Z])
            p4 = pt.tile([128, TW], BF16, name="p4")
            nc.gpsimd.tensor_add(p4[:, :Z], p4t[:, :Z], p4t[:, Z:2 * Z])
            nc.gpsimd.tensor_add(p4[:, :Z], p4[:, :Z], p4t2[:, :Z])

            # ---- GpS: p3 = v * bcast(c) -----------------------------------
            p3 = pt.tile([128, 3 * TW], BF16, name="p3")
            nc.gpsimd.tensor_mul(_ap3(p3[:, :3 * Z], 3, Z),
                                 _ap3(x_v[:, :3 * Z], 3, Z),
                                 _bcast(cw[:, :Z], 3))

            # ---- h6..h8 = [b_j0|b_j1] -> p2_j -----------------------------
            p2 = pt.tile([128, 6 * TW], BF16, name="p2")
            for j in range(3):
                h = grab()
                mm(h[:, 0:Z], w011[:, 0:128], vx(j))
                mm(h[:, Z:2 * Z], w011[:, 128:256], vx(j))
                nc.vector.tensor_mul(p2[:, j * 2 * Z:(j + 1) * 2 * Z],
                                     x_s[:, :2 * Z], h[:, :2 * Z])

            # ---- DVE 2x: A/B products (late; consumed next step) ----------
            # A = [v0E1|v1E2|v2E0] = [Az|Ax|Ay] (<=1024-col ops)
            A = mid.tile([128, 3 * TW], BF16, name="A")
            nc.vector.tensor_mul(A[:, 0:2 * Z], vx(0, 2), ew(0, 2))
            nc.vector.tensor_mul(A[:, 2 * Z:3 * Z], vx(2), ew(2))
            B = mid.tile([128, 3 * TW], BF16, name="B")
            nc.vector.tensor_mul(B[:, 0:Z], vx(2), ew(0))
            nc.vector.tensor_mul(B[:, Z:3 * Z], vx(0, 2), ew(1, 2))
            return dict(z0=z0, Z=Z, p1=p1, p4=p4, p2=p2, p3=p3, A=A, B=B)

        def stage2(st, split_store=False):
            z0, Z = st["z0"], st["Z"]
            p1, p4, p2, p3, A, B = (st["p1"], st["p4"], st["p2"], st["p3"],
                                    st["A"], st["B"])
            stor = ost.tile([128, 8 * TW], BF16, name="stor")

            def grab():
                return ps1.tile([128, 2 * TW], F32, name="s1", bufs=4)

            def Ak(k):
                # A stored rotated: [Az|Ax|Ay]
                return A[:, ((k + 1) % 3) * Z:(((k + 1) % 3) + 1) * Z]

            k1 = grab()
            for m in range(2):
                o = k1[:, m * Z:(m + 1) * Z]
                mm(o, L0e[0][:, 128 * m:128 * (m + 1)], p1[:, :Z],
                   start=True, stop=False)
                mm(o, L0e[1][:, 128 * m:128 * (m + 1)], p1[:, Z:2 * Z],
                   start=False, stop=False)
                mm(o, L0e[2][:, 128 * m:128 * (m + 1)], p4[:, :Z],
                   start=False, stop=True)
            nc.scalar.copy(stor[:, 0:2 * Z], k1[:, :2 * Z])
            if split_store:
                nc.scalar.dma_start(_dram3(outT_d, 0, 2, z0, Z),
                                    _ap3(stor[:, 0:2 * Z], 2, Z))
            k2 = grab()
            for j in range(2):
                o = k2[:, j * Z:(j + 1) * Z]
                mm(o, L1e2[:, 0:128], Ak(j), start=True, stop=False)
                mm(o, L1e2[:, 128:256], B[:, j * Z:(j + 1) * Z],
                   start=False, stop=True)
            nc.scalar.copy(stor[:, 2 * Z:4 * Z], k2[:, :2 * Z])
            if split_store:
                nc.sync.dma_start(_dram3(outT_d, 256, 2, z0, Z),
                                  _ap3(stor[:, 2 * Z:4 * Z], 2, Z))

            def o1o(o, j):
                mm(o, L1o[0][:, :], p2[:, j * 2 * Z:j * 2 * Z + Z],
                   start=True, stop=False)
                mm(o, L1o[1][:, :], p2[:, j * 2 * Z + Z:(j + 1) * 2 * Z],
                   start=False, stop=False)
                mm(o, L1o[2][:, :], p3[:, j * Z:(j + 1) * Z],
                   start=False, stop=True)

            k3 = grab()
            mm(k3[:, 0:Z], L1e2[:, 0:128], Ak(2), start=True, stop=False)
            mm(k3[:, 0:Z], L1e2[:, 128:256], B[:, 2 * Z:3 * Z],
               start=False, stop=True)
            o1o(k3[:, Z:2 * Z], 0)
            nc.scalar.copy(stor[:, 4 * Z:6 * Z], k3[:, :2 * Z])
            if split_store:
                nc.scalar.dma_start(_dram3(outT_d, 512, 2, z0, Z),
                                    _ap3(stor[:, 4 * Z:6 * Z], 2, Z))
            k4 = grab()
            o1o(k4[:, 0:Z], 1)
            o1o(k4[:, Z:2 * Z], 2)
            nc.scalar.copy(stor[:, 6 * Z:8 * Z], k4[:, :2 * Z])
            if split_store:
                nc.sync.dma_start(_dram3(outT_d, 768, 2, z0, Z),
                                  _ap3(stor[:, 6 * Z:8 * Z], 2, Z))
            else:
                # inline trigger on Sync: the idle engine absorbs the
                # DMA-ring backpressure waits (a scalar trigger would
                # block ACT's FIFO for ~17us -- measured in v6)
                nc.sync.dma_start(_dram3(outT_d, 0, 8, z0, Z),
                                  _ap3(stor[:, :8 * Z], 8, Z))

        # software pipeline with 2-deep load prefetch
        nblk = len(zblocks)
        pend1 = None
        for bi in range(nblk):
            st = stage1(bi, xq.pop(0))
            if pend1 is not None:
                stage2(pend1, split_store=(bi - 1 >= nblk - 2))
            pend1 = st
            if bi + 2 < nblk:
                xq.append(load(bi + 2))
        stage2(pend1, split_store=True)


def _prep_inputs(node_feat, w_00_0, w_01_1, w_10_1, w_11_0, w_11_1,
                 W_0e, W_1o, W_1e):
    import ml_dtypes
    ndt = ml_dtypes.bfloat16
    l1e = W_1e / np.sqrt(128.0)
    l1e2 = np.concatenate([l1e, -l1e], axis=1)        # [128, 256]
    wt000 = np.ascontiguousarray((C_000 * w_00_0).T)  # [256, 256]
    wt011 = np.ascontiguousarray((C_011 * w_01_1).T)  # [128, 256]
    wt101 = np.ascontiguousarray((C_101 * w_10_1).T)  # [256, 128]
    wt110 = np.ascontiguousarray((C_110 * w_11_0).T)  # [128, 128]
    wt111 = np.ascontiguousarray((C_111 * w_11_1).T)  # [128, 128]
    l0e = W_0e / np.sqrt(384.0)                       # [384, 256]
    l1o = W_1o / np.sqrt(384.0)                       # [384, 128]

    wall = np.zeros((128, WALL_COLS), np.float32)

    def put(key, arr):
        c0, w = WCOL[key]
        assert arr.shape == (128, w), (key, arr.shape)
        wall[:, c0:c0 + w] = arr

    put("w111", wt111)
    put("w101_0", wt101[0:128]); put("w101_1", wt101[128:256])
    put("w000_0", wt000[0:128]); put("w000_1", wt000[128:256])
    put("w110", wt110)
    put("w011", wt011)
    put("l0e_0", l0e[0:128]); put("l0e_1", l0e[128:256]); put("l0e_2", l0e[256:384])
    put("l1e2", l1e2)
    put("l1o_0", l1o[0:128]); put("l1o_1", l1o[128:256]); put("l1o_2", l1o[256:384])
    wall = wall.astype(ndt)

    feat = np.asarray(node_feat, dtype=np.float32).reshape(N_CORES, NS, 640)
    in_maps = []
    for i in range(N_CORES):
        blk = feat[i]
        xT = np.zeros((640, NSH), ndt)
        xT[:256, :NS] = blk[:, :256].T.astype(ndt)
        vv = blk[:, 256:].reshape(NS, 128, 3)
        xT[256:, :NS] = vv.transpose(2, 1, 0).reshape(384, NS).astype(ndt)
        in_maps.append({"xT": xT, "wall": wall})
    return in_maps


def _gather(results):
    # outT rows: [o0e(0:256) | o1e(256:640) | o1o(640:1024)]
    out = np.empty((N_NODES, 1024), np.float32)
    for i in range(N_CORES):
        oT = np.asarray(results[i]["outT"]).astype(np.float32,
                                                   copy=False)[:, :NS]
        blk = out[i * NS:(i + 1) * NS]
        blk[:, :256] = oT[:256].T
        blk[:, 640:] = oT[256:640].reshape(3, 128, NS).transpose(2, 1, 0) \
            .reshape(NS, 384)
        blk[:, 256:640] = oT[640:].reshape(3, 128, NS).transpose(2, 1, 0) \
            .reshape(NS, 384)
    return out


def kernel(node_feat, w_00_0, w_01_1, w_10_1, w_11_0, w_11_1,
           W_0e, W_1o, W_1e, _trace=False):
    if "k" not in _CACHE:
        _CACHE["k"] = _build_program()
    nc = _CACHE["k"]
    in_maps = _prep_inputs(node_feat, w_00_0, w_01_1, w_10_1, w_11_0,
                           w_11_1, W_0e, W_1o, W_1e)
    res = run_bass_kernel_spmd(nc, in_maps, core_ids=list(range(N_CORES)),
                               trace=_trace)
    out = _gather(res.results)
    if _trace:
        return out, res
    return out
